# revision 1
# baseline (speedup 1.0000x reference)
"""Trainium2 Bass kernel for the DriftingPolicy loss (8-core SPMD).

Math (value-equivalent to the reference):
  loss = mean(V_total^2) over [N, D], where for each temperature T in
  {0.05, 0.1, 0.2} (written as T = 0.2 / t_hat, t_hat in {1, 2, 4}):
    d[i, n]   = dist(x_i, y_n) over cols n = [y_neg | y_pos], diag of the
                neg block poisoned to a huge value (reference adds 1e6).
    K_t = exp(-t_hat * d / (0.2 * mean(d_pos)));  c_n = col sums
    K' = K / sqrt(c_n)
    V += (rn_i/s_i) * (K'_pos @ y_pos) - (rp_i/s_i) * (K'_neg @ y_neg)
       where rn_i = sum_neg K', rp_i = sum_pos K', s_i = sum_all K' * sqrt(c)

Sharding: rows of x strided across 8 cores (core c gets x[c::8]) so the
neg-block diagonal lands on a core-independent local pattern; y_pos/y_neg
replicated. Two all-reduce rounds: sum(d_pos) scalar, and per-temperature
column sums. Everything is computed in a column-major ("K transposed",
[n-partition, i-free]) layout so the second matmul needs no on-chip
transposes; host pre-transposes/casts the small inputs.

Engine split (v2): distances fold |x_i|^2 into the matmul (K=2 ones row
against a hi/lo bf16 split of |x|^2); ACT does sqrt (per chunk, with
|y_n|^2 bias + accum for the mean) and ONE fused full-tensor exp for the
base temperature E1; DVE derives the squared-temperature kernels from E1
(tensor_tensor_reduce chains for column sums, square+scale for K');
the hottest temperature (t_hat=4) instead re-exps from d on ACT to
balance engines.
"""

import sys

if "/opt/trn_rl_repo" not in sys.path:
    sys.path.insert(0, "/opt/trn_rl_repo")

import numpy as np
import ml_dtypes

import concourse.bass as bass
import concourse.mybir as mybir
import concourse.tile as tile
from concourse import bacc
from concourse.bass_utils import run_bass_kernel_spmd

F32 = mybir.dt.float32
F16 = mybir.dt.float16
BF16 = mybir.dt.bfloat16
AF = mybir.ActivationFunctionType
ALU = mybir.AluOpType

CORES = 8
N_FULL = 4096
D_FULL = 256
T_BASE = 0.2
T_HATS = (1.0, 2.0, 4.0)
POISON = 1.0e6  # added to dist^2 of neg-diagonal entries (-> exp underflows to 0)

D_DTYPE = F16


def build(cores=CORES, N=N_FULL, D=D_FULL, local_sim=False, repeat=1,
          no_poison=False, no_ttr=True, chunked_exp=True):
    # no_ttr=True: InstTensorTensorReduce hangs the device in this runtime;
    # use tensor_tensor + reduce_sum instead.
    """Builds the SPMD Bass kernel. Same NEFF runs on all cores.

    local_sim=True replaces collectives with local DMA copies so the module
    can run under single-core TimelineSim (timing analysis only).
    repeat>1 re-runs the whole computation that many times (for slope-based
    wall-clock timing: dispatch overhead cancels between repeat counts).
    """
    M = N // cores            # local rows per core
    NEGCH = N // 128          # neg column chunks
    NCH = 2 * NEGCH           # total column chunks (neg then pos)
    KCH = D // 128            # contraction chunks for the distance matmul
    WIN = 128 // cores        # poison window width per neg chunk
    ISUB = (M + 127) // 128   # 128-row output subchunks
    NT = len(T_HATS)
    assert M % 128 == 0 and D % 128 == 0 and N % 128 == 0 and M <= 512
    assert WIN * NEGCH == M

    nc = bacc.Bacc(
        "TRN2",
        target_bir_lowering=False,
        debug=False,
        enable_asserts=True,
        num_devices=cores,
    )

    # ---- kernel I/O ----
    xT2_d = nc.dram_tensor("xT2", [D, M], BF16, kind="ExternalInput")
    xse_d = nc.dram_tensor("xse", [128, M], BF16, kind="ExternalInput")
    yTn_d = nc.dram_tensor("yTn", [D, N], BF16, kind="ExternalInput")
    yTp_d = nc.dram_tensor("yTp", [D, N], BF16, kind="ExternalInput")
    yan_d = nc.dram_tensor("yan", [N, 258], BF16, kind="ExternalInput")
    yap_d = nc.dram_tensor("yap", [N, 258], BF16, kind="ExternalInput")
    yxn_d = nc.dram_tensor("yxn", [128, N], BF16, kind="ExternalInput")
    yxp_d = nc.dram_tensor("yxp", [128, N], BF16, kind="ExternalInput")
    poison_d = nc.dram_tensor("poison", [128, WIN], F32, kind="ExternalInput")
    ones_d = nc.dram_tensor("ones128", [128, 128], F32, kind="ExternalInput")
    loss_d = nc.dram_tensor("losspart", [128, 1], F32, kind="ExternalOutput")

    rg = [list(range(cores))]

    def all_reduce(inb, outb):
        if local_sim:
            nc.sync.dma_start(outb[:], inb[:])
        else:
            nc.gpsimd.collective_compute(
                "AllReduce",
                ALU.add,
                replica_groups=rg,
                ins=[inb[:].opt()],
                outs=[outb[:].opt()],
            )

    with tile.TileContext(nc) as tc:
        with (
            tc.tile_pool(name="consts", bufs=1) as consts,
            tc.tile_pool(name="stats", bufs=1) as stats,
            tc.tile_pool(name="dram", bufs=1, space="DRAM") as dram,
            tc.tile_pool(name="pbig", bufs=1) as pbig,
            tc.tile_pool(name="scr16", bufs=3) as scr16,
            tc.tile_pool(name="drain", bufs=3) as drain,
            tc.tile_pool(name="tstat", bufs=2) as tstat,
        ):
            # ---- load constants (resident for the whole kernel) ----
            xT2 = consts.tile([128, KCH, M], BF16, name="xT2_sb")
            nc.sync.dma_start(xT2[:], xT2_d[:].rearrange("(k p) f -> p k f", p=128))
            xse = consts.tile([128, M], BF16, name="xse_sb")
            nc.sync.dma_start(xse[:], xse_d[:])
            yx = []
            for h, src_ in enumerate((yxn_d, yxp_d)):
                t = consts.tile([128, N], BF16, name=f"yx_sb{h}")
                nc.sync.dma_start(t[:], src_[:])
                yx.append(t)
            ya = []
            for h, src in enumerate((yan_d, yap_d)):
                t = consts.tile([128, NEGCH, 258], BF16, name=f"ya_sb{h}")
                nc.sync.dma_start(t[:], src[:].rearrange("(c p) f -> p c f", p=128))
                ya.append(t)
            poisonT = consts.tile([128, WIN], F32, name="poison_sb")
            nc.sync.dma_start(poisonT[:], poison_d[:])
            ones128 = consts.tile([128, 128], F32, name="ones_sb")
            nc.sync.dma_start(ones128[:], ones_d[:])

            # ---- persistent state ----
            dsum = stats.tile([128, NEGCH], F32, name="dsum")
            scales = stats.tile([128, NT], F32, name="scales")
            colp = [stats.tile([128, NCH], F32, name=f"colp{t}") for t in range(NT)]
            colg = [stats.tile([128, NCH], F32, name=f"colg{t}") for t in range(NT)]
            V_sb = stats.tile([128, ISUB, D], F32, name="V_sb")
            lp = stats.tile([128, ISUB], F32, name="lp")
            msum = stats.tile([128, 1], F32, name="msum")
            sc_vec = stats.tile([128, NT], F32, name="sc_vec")
            inv_s = stats.tile([1, 1], F32, name="inv_s")
            s_sc = stats.tile([1, 1], F32, name="s_sc")
            dtot = stats.tile([128, 1], F32, name="dtot")
            lout = stats.tile([128, 1], F32, name="lout")

            for rep in range(repeat):
                # DRAM bounce buffers for collectives (a Shared output may
                # only be written by a single instruction -> per-rep tiles)
                mean_in = dram.tile([128, 1], F32, name=f"mean_in{rep}")
                mean_out = dram.tile(
                    [128, 1], F32, name=f"mean_out{rep}", addr_space="Shared"
                )
                col_in = [
                    dram.tile([128, NCH], F32, name=f"col_in{t}_{rep}")
                    for t in range(NT)
                ]
                col_out = [
                    dram.tile(
                        [128, NCH], F32, name=f"col_out{t}_{rep}",
                        addr_space="Shared",
                    )
                    for t in range(NT)
                ]

                GRP = min(8, NCH)

                # Slot sharing (pool tags): d and e2 share "slotA";
                # yT and e1 share "slotB". Tile serializes via deps, the
                # allocator reuses the space.
                d_sb = pbig.tile([128, NCH, M], D_DTYPE, name=f"d_sb{rep}",
                                 tag="slotA")

                # ================= phase A: distances =================
                with (
                    tc.tile_pool(name=f"pa{rep}", bufs=2, space="PSUM") as pa,
                ):
                    def load_yT(h):
                        t = pbig.tile([128, KCH, N], BF16, name="yT_sb",
                                      tag="slotB")
                        nc.sync.dma_start(
                            t[:],
                            (yTp_d if h else yTn_d)[:].rearrange(
                                "(k p) f -> p k f", p=128
                            ),
                        )
                        return t

                    GA = min(2, NEGCH)  # chunks per fused-sqrt group

                    def do_group(g, yT):
                        # chunks [g*GA, (g+1)*GA), all in the same half
                        c0 = g * GA
                        pos = c0 >= NEGCH
                        ps = pa.tile([128, GA, M], F32, name="ps_d")
                        for j in range(GA):
                            c = c0 + j
                            cl = c - NEGCH if pos else c
                            for k in range(KCH):
                                nc.tensor.matmul(
                                    ps[:, j, :],
                                    yT[:, k, cl * 128 : (cl + 1) * 128],
                                    xT2[:, k, :],
                                    start=(k == 0),
                                    stop=False,
                                )
                            # |x|^2 and |y|^2 via hi/lo bf16 ones rows
                            nc.tensor.matmul(
                                ps[:, j, :],
                                yx[1 if pos else 0][:, cl * 128 : (cl + 1) * 128],
                                xse[:],
                                start=False,
                                stop=True,
                            )
                            if not pos and not no_poison:
                                nc.vector.tensor_tensor(
                                    ps[:, j, cl * WIN : (cl + 1) * WIN],
                                    ps[:, j, cl * WIN : (cl + 1) * WIN],
                                    poisonT[:],
                                    ALU.add,
                                )
                        gp = g - NEGCH // GA if pos else None
                        nc.scalar.activation(
                            d_sb[:, c0 : c0 + GA, :],
                            ps[:],
                            AF.Sqrt,
                            accum_out=dsum[:, gp : gp + 1] if pos else None,
                        )

                    # pos groups first: they feed the mean all-reduce
                    yt = load_yT(1)
                    for g in range(NEGCH // GA, NCH // GA):
                        do_group(g, yt)

                    # mean all-reduce (overlaps with the neg-chunk work below)
                    nc.vector.reduce_sum(dtot[:], dsum[:, 0 : NEGCH // GA], axis=mybir.AxisListType.X)
                    nc.sync.dma_start(mean_in[:], dtot[:])
                    all_reduce(mean_in, mean_out)
                    nc.sync.dma_start(msum[:], mean_out[:])

                    yt = load_yT(0)
                    for g in range(0, NEGCH // GA):
                        do_group(g, yt)

                    # ---- scales from the mean ----
                    with tc.tile_pool(
                        name=f"psmall{rep}", bufs=1, space="PSUM"
                    ) as psmall:
                        ps1 = psmall.tile([1, 1], F32, name="ps1")
                        nc.tensor.matmul(
                            ps1[:], msum[:], ones128[:, 0:1], start=True, stop=True
                        )
                        nc.scalar.copy(s_sc[:], ps1[:])
                        nc.vector.reciprocal(inv_s[:], s_sc[:])
                        nc.vector.memset(sc_vec[:], 0.0)
                        for t, th in enumerate(T_HATS):
                            coef = -th * (N * N) / T_BASE
                            nc.vector.tensor_scalar_mul(
                                sc_vec[0:1, t : t + 1], inv_s[0:1, 0:1], coef
                            )
                        psb = psmall.tile([128, NT], F32, name="psb")
                        nc.tensor.matmul(
                            psb[:], ones128[:], sc_vec[0:128, :], start=True,
                            stop=True,
                        )
                        nc.scalar.copy(scales[:], psb[:])

                # ============ phase B1: base exp + its column sums ============
                e1_sb = pbig.tile([128, NCH, M], BF16, name=f"e1_sb{rep}",
                                  tag="slotB")
                for g in range(0, NCH, GRP):
                    nc.scalar.activation(
                        e1_sb[:, g : g + GRP, :],
                        d_sb[:, g : g + GRP, :],
                        AF.Exp,
                        bias=0.0,
                        scale=scales[:, 0:1],
                    )

                def col_accum(src_sb, t):
                    for c in range(NCH):
                        cs = scr16.tile([128, M], BF16, name="cs_scr", tag="cs")
                        nc.vector.tensor_scalar(
                            cs[:],
                            src_sb[:, c, :],
                            1.0,
                            0.0,
                            ALU.mult,
                            ALU.add,
                            accum_out=colp[t][:, c : c + 1],
                        )

                def launch_ar(t):
                    nc.sync.dma_start(col_in[t][:], colp[t][:])
                    all_reduce(col_in[t], col_out[t])
                    nc.sync.dma_start(colg[t][:], col_out[t][:])

                col_accum(e1_sb, 0)
                launch_ar(0)

                with (
                    tc.tile_pool(name=f"pc{rep}", bufs=1, space="PSUM") as pc,
                ):
                    e2_sb = pbig.tile([128, NCH, M], BF16, name=f"e2_sb{rep}",
                                      tag="slotA")

                    def scale_ya(t):
                        # ic = 1/sqrt(c); scale y-side rows (cols 0..256) by it
                        rc = tstat.tile([128, NCH], F32, name="rc", tag="rc")
                        nc.vector.reciprocal(rc[:], colg[t][:])
                        ict = tstat.tile([128, NCH], F32, name="ict", tag="ict")
                        nc.scalar.activation(ict[:], rc[:], AF.Sqrt)
                        for h, src in enumerate((yan_d, yap_d)):
                            if t > 0 or rep > 0:
                                nc.sync.dma_start(
                                    ya[h][:],
                                    src[:].rearrange("(c p) f -> p c f", p=128),
                                )
                            nc.vector.tensor_tensor(
                                ya[h][:, :, 0:257],
                                ya[h][:, :, 0:257],
                                ict[
                                    :, h * NEGCH : (h + 1) * NEGCH, None
                                ].to_broadcast((128, NEGCH, 257)),
                                ALU.mult,
                            )

                    def mm_temp(t, kp_of_chunk):
                        psums = [
                            [
                                pc.tile(
                                    [128, 258],
                                    F32,
                                    name=f"pch{h}_{i}",
                                    tag=f"pch{h}_{i}",
                                )
                                for i in range(ISUB)
                            ]
                            for h in range(2)
                        ]
                        for c in range(NCH):
                            pos = c >= NEGCH
                            cl = c - NEGCH if pos else c
                            kp = kp_of_chunk(c)
                            for i in range(ISUB):
                                nc.tensor.matmul(
                                    psums[1 if pos else 0][i][:],
                                    kp[:, i * 128 : (i + 1) * 128],
                                    ya[1 if pos else 0][:, cl, :],
                                    start=(cl == 0),
                                    stop=(cl == NEGCH - 1),
                                )
                        for i in range(ISUB):
                            pn, pp = psums[0][i], psums[1][i]
                            rn_s = drain.tile([128, 2], F32, name="rn_s")
                            rp_s = drain.tile([128, 2], F32, name="rp_s")
                            nc.vector.tensor_copy(rn_s[:], pn[:, 256:258])
                            nc.vector.tensor_copy(rp_s[:], pp[:, 256:258])
                            st = drain.tile([128, 1], F32, name="st")
                            nc.vector.tensor_tensor(
                                st[:], rn_s[:, 1:2], rp_s[:, 1:2], ALU.add
                            )
                            rinv = drain.tile([128, 1], F32, name="rinv")
                            nc.vector.reciprocal(rinv[:], st[:])
                            af = drain.tile([128, 1], F32, name="af")
                            bf = drain.tile([128, 1], F32, name="bf")
                            nc.vector.tensor_tensor(
                                af[:], rn_s[:, 0:1], rinv[:], ALU.mult
                            )
                            nc.vector.tensor_tensor(
                                bf[:], rp_s[:, 0:1], rinv[:], ALU.mult
                            )
                            u1 = drain.tile([128, D], F32, name="u1")
                            u2 = drain.tile([128, D], F32, name="u2")
                            nc.vector.tensor_scalar_mul(u1[:], pp[:, 0:D], af[:])
                            nc.vector.tensor_scalar_mul(u2[:], pn[:, 0:D], bf[:])
                            if t == 0:
                                nc.vector.tensor_tensor(
                                    V_sb[:, i, :], u1[:], u2[:], ALU.subtract
                                )
                            else:
                                nc.vector.tensor_tensor(
                                    V_sb[:, i, :], V_sb[:, i, :], u1[:], ALU.add
                                )
                                nc.vector.tensor_tensor(
                                    V_sb[:, i, :], V_sb[:, i, :], u2[:],
                                    ALU.subtract,
                                )

                    # ---- temp 0 first (overlaps the e2/e4 chains below) ----
                    scale_ya(0)
                    mm_temp(0, lambda c: e1_sb[:, c, :])

                    # ---- e2 = e1^2 (fused, ACT) + its column sums ----
                    for g in range(0, NCH, GRP):
                        nc.scalar.activation(
                            e2_sb[:, g : g + GRP, :],
                            e1_sb[:, g : g + GRP, :],
                            AF.Square,
                        )
                    col_accum(e2_sb, 1)
                    launch_ar(1)
                    scale_ya(1)
                    mm_temp(1, lambda c: e2_sb[:, c, :])

                    # ---- e4 col sums: ACT Square(e2) with fused accum ----
                    for c in range(NCH):
                        e4 = scr16.tile([128, M], BF16, name="e4_scr", tag="e4")
                        nc.scalar.activation(
                            e4[:],
                            e2_sb[:, c, :],
                            AF.Square,
                            accum_out=colp[2][:, c : c + 1],
                        )
                    launch_ar(2)
                    scale_ya(2)

                    def kp4(c):
                        kpt = scr16.tile([128, M], BF16, name="kp_scr", tag="kp")
                        nc.scalar.activation(kpt[:], e2_sb[:, c, :], AF.Square)
                        return kpt[:]

                    mm_temp(2, kp4)

                # ---- loss partials ----
                for i in range(ISUB):
                    scr = drain.tile([128, D], F32, name="sq_scr")
                    nc.scalar.activation(
                        scr[:],
                        V_sb[:, i, :],
                        AF.Square,
                        accum_out=lp[:, i : i + 1],
                    )
                nc.vector.reduce_sum(lout[:], lp[:], axis=mybir.AxisListType.X)
                nc.sync.dma_start(loss_d[:], lout[:])

    nc.compile()
    return nc


def prepare_inputs(x, y_pos, y_neg, cores=CORES):
    """Host-side input prep: shard, transpose, cast, norms, masks."""
    x = np.asarray(x, dtype=np.float32)
    y_pos = np.asarray(y_pos, dtype=np.float32)
    y_neg = np.asarray(y_neg, dtype=np.float32)
    N, D = x.shape
    M = N // cores
    NEGCH = N // 128
    WIN = 128 // cores
    bf = ml_dtypes.bfloat16

    def aug(y):
        a = np.zeros((N, 258), dtype=bf)
        a[:, :D] = y.astype(bf)
        a[:, 256] = bf(1.0)  # -> rn/rp (gets the ic scaling)
        a[:, 257] = bf(1.0)  # -> s_i (stays unscaled)
        return a

    def yxmat(y):
        s = (y * y).sum(axis=1).astype(np.float32)  # [N]
        hi = s.astype(bf)
        lo = (s - hi.astype(np.float32)).astype(bf)
        m = np.zeros((128, N), dtype=bf)
        m[0] = bf(1.0)
        m[1] = bf(1.0)
        m[2] = hi
        m[3] = lo
        return m

    shared = {
        "yTn": np.ascontiguousarray(y_neg.T).astype(bf),
        "yTp": np.ascontiguousarray(y_pos.T).astype(bf),
        "yan": aug(y_neg),
        "yap": aug(y_pos),
        "yxn": yxmat(y_neg),
        "yxp": yxmat(y_pos),
        "ones128": np.ones((128, 128), dtype=np.float32),
    }
    in_maps = []
    for c in range(cores):
        xs = x[c::cores]  # [M, D]
        sqx = (xs * xs).sum(axis=1).astype(np.float32)  # [M]
        hi = sqx.astype(bf)
        lo = (sqx - hi.astype(np.float32)).astype(bf)
        xse = np.zeros((128, M), dtype=bf)
        xse[0] = hi
        xse[1] = lo
        xse[2] = bf(1.0)
        xse[3] = bf(1.0)
        poison = np.zeros((128, WIN), dtype=np.float32)
        for q in range(WIN):
            poison[c + cores * q, q] = POISON
        m = dict(shared)
        m["xT2"] = np.ascontiguousarray((-2.0 * xs).T).astype(bf)
        m["xse"] = xse
        m["poison"] = poison
        in_maps.append(m)
    return in_maps


_CACHED = {}


def _get_nc(cores, N, D, repeat=1):
    key = (cores, N, D, repeat)
    if key not in _CACHED:
        _CACHED[key] = build(cores, N, D, repeat=repeat)
    return _CACHED[key]


def kernel(x, y_pos, y_neg, _trace=False, _tracekw=None):
    x = np.asarray(x)
    N, D = x.shape
    nc = _get_nc(CORES, N, D)
    in_maps = prepare_inputs(x, y_pos, y_neg, CORES)
    kw = dict(_tracekw or {})
    res = run_bass_kernel_spmd(
        nc, in_maps, core_ids=list(range(CORES)), trace=_trace, **kw
    )
    total = sum(float(res.results[c]["losspart"].sum()) for c in range(CORES))
    loss = np.float32(total / (N * D))
    out = np.array(loss, dtype=np.float32)
    if _trace:
        return out, res
    return out


if __name__ == "__main__":
    rng = np.random.default_rng(0)
    N, D = N_FULL, D_FULL
    x = rng.standard_normal((N, D)).astype(np.float32)
    yp = rng.standard_normal((N, D)).astype(np.float32)
    yn = rng.standard_normal((N, D)).astype(np.float32)
    print("loss:", kernel(x, yp, yn))



# revision 3
# speedup vs baseline: 1.1152x; 1.1152x over previous
"""Trainium2 Bass kernel for the DriftingPolicy loss (8-core SPMD), v2.

Math (identical to the baseline / reference):
  For T in {0.2, 0.1, 0.05} = 0.2 / t_hat, t_hat in {1, 2, 4}:
    K_t[n, i] = exp(-t_hat * d[n, i] * N^2 / (0.2 * S)),  S = global sum(d_pos)
    c_n^t  = global column sums of K_t (over all rows i)   -> all-reduce
    K'_t   = K_t / sqrt(c^t);  rn_i = sum_neg K'_t, rp_i = sum_pos K'_t,
    r_i    = sum_all K_t (raw)
    V_i   += (rn_i/r_i) * (K'_pos @ y_pos)_i - (rp_i/r_i) * (K'_neg @ y_neg)_i
  loss = mean(V^2)

v2 restructure vs baseline (~375us -> ~335us measured; most of the
remaining span is cross-core launch skew absorbed at the first collective):
  * Chunk-granular pipeline; the t_hat=1 column-sum AllReduce is split
    pos/neg so the second-phase matmuls start on the pos half early.
  * sqrt writes f16 into e1_sb and exp runs IN PLACE per chunk: no d
    buffer, and crucially no d-slot WAR chain forcing sqrt/exp interleave
    on ACT (which cost ~40 ACT table reloads at 1.3us each in v1 drafts).
  * Column sums fused into the producers via accum_out: colsum1 in the
    per-chunk ACT exp, colsum2 in the DVE scalar_tensor_tensor that forms
    e2 = e1*e1, colsum4 split ACT(pos)/DVE(neg) over a scratch.
  * ya (augmented y) streamed from DRAM in 8-chunk batches and scaled by
    1/sqrt(c) in a small rotating pool; 1/sqrt(c) computed on DVE with the
    bit-trick rsqrt + 2 Newton steps (ACT Rsqrt/Ln would thrash tables,
    DVE pow fails the ISA check).
  * Scales matmuls emitted after the neg distance matmuls so the in-order
    PE queue is not head-blocked waiting on the mean AllReduce.
  * All host-side layouts are partition-contiguous (>=516B descriptors).
  * V updates via scalar_tensor_tensor accumulation directly into V_sb.

Column-chunk order is POS first (chunks 0..31), NEG second (32..63), so the
mean(dist_pos) all-reduce and the first column-sum all-reduce both fire as
early as possible.
"""

import sys

if "/opt/trn_rl_repo" not in sys.path:
    sys.path.insert(0, "/opt/trn_rl_repo")

import numpy as np
import ml_dtypes

import concourse.bass as bass
import concourse.mybir as mybir
import concourse.tile as tile
from concourse import bacc
from concourse.bass_utils import run_bass_kernel_spmd
from concourse.tile_rust import add_dep_helper

F32 = mybir.dt.float32
F16 = mybir.dt.float16
BF16 = mybir.dt.bfloat16
AF = mybir.ActivationFunctionType
ALU = mybir.AluOpType

CORES = 8
N_FULL = 4096
D_FULL = 256
T_BASE = 0.2
T_HATS = (1.0, 2.0, 4.0)
POISON = 1.0e6

D_DTYPE = F16


def build(cores=CORES, N=N_FULL, D=D_FULL, local_sim=False,
          colsum_engine="gpsimd", e4mat_engine="dve", yas_bufs=12,
          d_bufs=16):
    """Builds the SPMD Bass kernel. Same NEFF runs on all cores."""
    M = N // cores            # local rows per core (512)
    HCH = N // 128            # chunks per half (32)
    NCH = 2 * HCH             # total column chunks (pos then neg) (64)
    KCH = D // 128            # contraction chunks (2)
    WIN = 128 // cores        # poison window width per neg chunk (16)
    ISUB = M // 128           # 128-row output subchunks (4)
    NT = len(T_HATS)
    GA = 2                    # chunks per sqrt/psum group
    assert M % 128 == 0 and D % 128 == 0 and N % 128 == 0 and M <= 512
    assert WIN * HCH == M

    nc = bacc.Bacc(
        "TRN2",
        target_bir_lowering=False,
        debug=False,
        enable_asserts=True,
        num_devices=cores,
    )

    # ---- kernel I/O (all host-prepped, partition-contiguous) ----
    xT2_d = nc.dram_tensor("xT2", [128, KCH * M], BF16, kind="ExternalInput")
    xse_d = nc.dram_tensor("xse", [128, M], BF16, kind="ExternalInput")
    yTp_d = nc.dram_tensor("yTp", [128, KCH * N], BF16, kind="ExternalInput")
    yTn_d = nc.dram_tensor("yTn", [128, KCH * N], BF16, kind="ExternalInput")
    yap_d = nc.dram_tensor("yap", [128, HCH * 258], BF16, kind="ExternalInput")
    yan_d = nc.dram_tensor("yan", [128, HCH * 258], BF16, kind="ExternalInput")
    yxp_d = nc.dram_tensor("yxp", [128, N], BF16, kind="ExternalInput")
    yxn_d = nc.dram_tensor("yxn", [128, N], BF16, kind="ExternalInput")
    poison_d = nc.dram_tensor("poison", [128, WIN], F32, kind="ExternalInput")
    ones_d = nc.dram_tensor("ones128", [128, 128], F32, kind="ExternalInput")
    loss_d = nc.dram_tensor("losspart", [128, 1], F32, kind="ExternalOutput")

    rg = [list(range(cores))]

    def all_reduce(inb, outb):
        if local_sim:
            nc.sync.dma_start(outb[:], inb[:])
        else:
            nc.gpsimd.collective_compute(
                "AllReduce",
                ALU.add,
                replica_groups=rg,
                ins=[inb[:].opt()],
                outs=[outb[:].opt()],
            )

    with tile.TileContext(nc) as tc:
        with (
            tc.tile_pool(name="consts", bufs=1) as consts,
            tc.tile_pool(name="stats", bufs=1) as stats,
            tc.tile_pool(name="dram", bufs=1, space="DRAM") as dram,
            tc.tile_pool(name="pbig", bufs=1) as pbig,
            tc.tile_pool(name="ytpool", bufs=2) as ytpool,
            tc.tile_pool(name="yaspool", bufs=3) as yaspool,
            tc.tile_pool(name="cspool", bufs=3) as cspool,
            tc.tile_pool(name="drain", bufs=4) as drain,
        ):
            # ---- constants ----
            xT2 = consts.tile([128, KCH, M], BF16, name="xT2_sb")
            nc.sync.dma_start(xT2[:], xT2_d[:].rearrange("p (k f) -> p k f", k=KCH))
            xse = consts.tile([128, M], BF16, name="xse_sb")
            nc.sync.dma_start(xse[:], xse_d[:])
            # yx loaded per half into one rotating slot (pos used first)
            yx = []
            for h, src_ in enumerate((yxp_d, yxn_d)):
                t = consts.tile([128, N], BF16, name=f"yx_sb{h}", tag="yx")
                nc.sync.dma_start(t[:], src_[:])
                yx.append(t)
            poisonT = consts.tile([128, WIN], F32, name="poison_sb")
            nc.sync.dma_start(poisonT[:], poison_d[:])
            ones128 = consts.tile([128, 128], F32, name="ones_sb")
            nc.sync.dma_start(ones128[:], ones_d[:])

            # ---- persistent state ----
            dsum = stats.tile([128, HCH // GA], F32, name="dsum")
            scales = stats.tile([128, NT], F32, name="scales")
            # col sums: t0 split pos/neg, t1/t2 whole
            colp = [
                stats.tile([128, HCH], F32, name="colp0a"),
                stats.tile([128, HCH], F32, name="colp0b"),
                stats.tile([128, NCH], F32, name="colp1"),
                stats.tile([128, NCH], F32, name="colp2"),
            ]
            colg = [
                stats.tile([128, HCH], F32, name="colg0a"),
                stats.tile([128, HCH], F32, name="colg0b"),
                stats.tile([128, NCH], F32, name="colg1"),
                stats.tile([128, NCH], F32, name="colg2"),
            ]
            # 1/sqrt(c) per temp: t0 pos, t0 neg, t1, t2
            icts = [
                stats.tile([128, HCH], F32, name="ict0a"),
                stats.tile([128, HCH], F32, name="ict0b"),
                stats.tile([128, NCH], F32, name="ict1"),
                stats.tile([128, NCH], F32, name="ict2"),
            ]
            V_sb = stats.tile([128, ISUB, D], F32, name="V_sb")
            msum = stats.tile([128, 1], F32, name="msum")
            sc_vec = stats.tile([128, NT], F32, name="sc_vec")
            inv_s = stats.tile([1, 1], F32, name="inv_s")
            s_sc = stats.tile([1, 1], F32, name="s_sc")
            dtot = stats.tile([128, 1], F32, name="dtot")
            lout = stats.tile([128, 1], F32, name="lout")

            nc.vector.memset(V_sb[:], 0.0)

            # DRAM bounce buffers for collectives
            mean_in = dram.tile([128, 1], F32, name="mean_in")
            mean_out = dram.tile([128, 1], F32, name="mean_out",
                                 addr_space="Shared")
            col_in = [
                dram.tile([128, HCH], F32, name="col_in0a"),
                dram.tile([128, HCH], F32, name="col_in0b"),
                dram.tile([128, NCH], F32, name="col_in1"),
                dram.tile([128, NCH], F32, name="col_in2"),
            ]
            col_out = [
                dram.tile([128, HCH], F32, name="col_out0a", addr_space="Shared"),
                dram.tile([128, HCH], F32, name="col_out0b", addr_space="Shared"),
                dram.tile([128, NCH], F32, name="col_out1", addr_space="Shared"),
                dram.tile([128, NCH], F32, name="col_out2", addr_space="Shared"),
            ]

            # ---- big tensors ----
            # e1 holds sqrt(d2) first (f16), then exp overwrites it chunk by
            # chunk IN PLACE -- no separate d buffer, no d-slot WAR chain
            # forcing sqrt/exp interleave on ACT.  e4 reuses e1's slot later;
            # e2 has its own slot.
            e1_sb = pbig.tile([128, NCH, M], F16, name="e1_sb", tag="slotB")
            e2_sb = pbig.tile([128, NCH, M], BF16, name="e2_sb", tag="slotC")

            # ================= phase A: distances =================
            sqrt_insts = []
            with tc.tile_pool(name="pa", bufs=3, space="PSUM") as pa, \
                 tc.tile_pool(name="psmall", bufs=1, space="PSUM") as psmall:

                NPART = 4                 # yT loads per half
                PW = N // NPART           # columns per yT load (1024)

                def load_yT(h, part):
                    t = ytpool.tile([128, KCH, PW], BF16, name="yT_sb",
                                    tag="yT")
                    src = (yTp_d, yTn_d)[h]
                    nc.sync.dma_start(
                        t[:],
                        src[:].rearrange("p (k f) -> p k f", k=KCH)[
                            :, :, part * PW: (part + 1) * PW
                        ],
                    )
                    return t

                def do_group(g, yT, part):
                    # chunks [g*GA, (g+1)*GA) within one half
                    c0 = g * GA
                    pos = c0 < HCH
                    ps = pa.tile([128, GA, M], F32, name="ps_d")
                    for j in range(GA):
                        c = c0 + j
                        cl = c if pos else c - HCH       # chunk within half
                        cc = cl - part * (PW // 128)     # chunk within yT part
                        for k in range(KCH):
                            nc.tensor.matmul(
                                ps[:, j, :],
                                yT[:, k, cc * 128: (cc + 1) * 128],
                                xT2[:, k, :],
                                start=(k == 0),
                                stop=False,
                            )
                        nc.tensor.matmul(
                            ps[:, j, :],
                            yx[0 if pos else 1][:, cl * 128: (cl + 1) * 128],
                            xse[:],
                            start=False,
                            stop=True,
                        )
                        if not pos:
                            nc.vector.tensor_tensor(
                                ps[:, j, cl * WIN: (cl + 1) * WIN],
                                ps[:, j, cl * WIN: (cl + 1) * WIN],
                                poisonT[:],
                                ALU.add,
                            )
                    si = nc.scalar.activation(
                        e1_sb[:, c0: c0 + GA, :],
                        ps[:],
                        AF.Sqrt,
                        accum_out=dsum[:, g: g + 1] if pos else None,
                    )
                    sqrt_insts.append(si)

                # pos half first: feeds the mean all-reduce
                npart = (PW // 128) // GA  # groups per yT part (4)
                for part in range(NPART):
                    yt = load_yT(0, part)
                    for g in range(part * npart, (part + 1) * npart):
                        do_group(g, yt, part)

                # mean all-reduce (overlaps the neg-half work below)
                nc.vector.reduce_sum(dtot[:], dsum[:],
                                     axis=mybir.AxisListType.X)
                nc.sync.dma_start(mean_in[:], dtot[:])
                all_reduce(mean_in, mean_out)
                nc.sync.dma_start(msum[:], mean_out[:])

                for part in range(NPART):
                    yt = load_yT(1, part)
                    for g in range(HCH // GA + part * npart,
                                   HCH // GA + (part + 1) * npart):
                        do_group(g, yt, part)

                # scales from the global mean.  Emitted AFTER the neg
                # distance matmuls: the ps1 matmul waits ~20us on the mean
                # all-reduce, and the in-order PE queue would head-block
                # every later matmul behind it.
                ps1 = psmall.tile([1, 1], F32, name="ps1")
                nc.tensor.matmul(ps1[:], msum[:], ones128[:, 0:1],
                                 start=True, stop=True)
                nc.vector.reciprocal(inv_s[:], ps1[:])
                nc.vector.memset(sc_vec[:], 0.0)
                for t, th in enumerate(T_HATS):
                    # local_sim: collectives are identity, so the "global"
                    # d_pos sum is the 1-core partial; shrink N^2 to match.
                    nn = (N * N) // cores if local_sim else N * N
                    coef = -th * nn / T_BASE
                    nc.vector.tensor_scalar_mul(
                        sc_vec[0:1, t: t + 1], inv_s[0:1, 0:1], coef
                    )
                psb = psmall.tile([128, NT], F32, name="psb")
                nc.tensor.matmul(psb[:], ones128[:], sc_vec[0:128, :],
                                 start=True, stop=True)
                nc.scalar.copy(scales[:], psb[:])

            # ============ exp / squares / column sums ============
            I32 = mybir.dt.int32

            def rsqrt_dve(dst, src, W):
                """dst = src^-0.5 on DVE only: bit-trick seed + 2 Newton
                steps.  Keeps the rsqrt off ACT (table reloads mid-exp) and
                off unsupported ALU ops (pow fails the DVE ISA check)."""
                sh = drain.tile([128, W], I32, name="rs_sh", tag="rs_sh")
                nc.vector.tensor_scalar(
                    sh[:], src.bitcast(I32), 1, None,
                    ALU.logical_shift_right,
                )
                y = drain.tile([128, W], F32, name="rs_y", tag="rs_y")
                nc.vector.tensor_scalar(
                    y[:].bitcast(I32), sh[:], -1, 0x5F3759DF,
                    ALU.mult, ALU.add,
                )
                t1 = drain.tile([128, W], F32, name="rs_t1", tag="rs_t1")
                for _ in range(2):
                    nc.vector.tensor_tensor(t1[:], y[:], y[:], ALU.mult)
                    nc.vector.tensor_tensor(t1[:], t1[:], src, ALU.mult)
                    nc.vector.tensor_scalar(
                        t1[:], t1[:], -0.5, 1.5, ALU.mult, ALU.add
                    )
                    nc.vector.tensor_tensor(y[:], y[:], t1[:], ALU.mult)
                nc.vector.tensor_copy(dst, y[:])

            def launch_ar(idx):
                nc.sync.dma_start(col_in[idx][:], colp[idx][:])
                all_reduce(col_in[idx], col_out[idx])
                nc.sync.dma_start(colg[idx][:], col_out[idx][:])
                W = HCH if idx < 2 else NCH
                rsqrt_dve(icts[idx][:], colg[idx][:], W)

            # e1 = exp(scale * d) per chunk on ACT, colsum1 fused via accum;
            # e2 = e1*e1 per chunk on DVE (scalar_tensor_tensor), colsum2
            # fused; e4 scratch per chunk on ACT Square, colsum4 fused.
            # (Pool engine rejects TensorScalarPtr, so no gpsimd colsums.)
            for c in range(NCH):
                cp = colp[0] if c < HCH else colp[1]
                cc = c if c < HCH else c - HCH
                ei = nc.scalar.activation(
                    e1_sb[:, c, :],
                    e1_sb[:, c, :],
                    AF.Exp,
                    bias=0.0,
                    scale=scales[:, 0:1],
                    accum_out=cp[:, cc: cc + 1],
                )
                # NOTE: no ordering edges needed -- with exp running in-place
                # on e1_sb there is no d-slot reuse chain forcing sqrt/exp
                # interleave, and the scheduler orders sqrt* then exp*
                # naturally (verified: 2 table loads total in the BIR).
                nc.vector.scalar_tensor_tensor(
                    e2_sb[:, c, :],
                    e1_sb[:, c, :],
                    1.0,
                    e1_sb[:, c, :],
                    ALU.mult,
                    ALU.mult,
                    accum_out=colp[2][:, c: c + 1],
                )
                if c == HCH - 1:
                    launch_ar(0)
                if c == NCH - 1:
                    launch_ar(1)
            launch_ar(2)

            # e4 column sums into a rotating scratch: pos half on ACT
            # (Square+accum), neg half on DVE (stt e2*e2 + accum) -- the two
            # streams run in parallel so colsum4 (the last AR) lands ~20us
            # earlier than a single-engine pass would.
            for c in range(NCH):
                if c < HCH:
                    sc4 = cspool.tile([128, M], BF16, name="e4_scr", tag="cs")
                    nc.scalar.activation(
                        sc4[:],
                        e2_sb[:, c, :],
                        AF.Square,
                        accum_out=colp[3][:, c: c + 1],
                    )
                else:
                    sc4 = cspool.tile([128, M], BF16, name="e4_scrd",
                                      tag="cs4d")
                    nc.vector.scalar_tensor_tensor(
                        sc4[:],
                        e2_sb[:, c, :],
                        1.0,
                        e2_sb[:, c, :],
                        ALU.mult,
                        ALU.mult,
                        accum_out=colp[3][:, c: c + 1],
                    )
            launch_ar(3)

            # ============ second-phase matmuls ============
            with tc.tile_pool(name="pc", bufs=1, space="PSUM") as pc:
                psums = [
                    [
                        pc.tile([128, 258], F32, name=f"pch{h}_{i}",
                                tag=f"pch{h}_{i}")
                        for i in range(ISUB)
                    ]
                    for h in range(2)
                ]

                YB = 8   # ya chunks per DMA batch

                def ya_batch(t, c0):
                    """Stream + scale YB augmented-y chunks for temp t."""
                    pos = c0 < HCH
                    cl0 = c0 if pos else c0 - HCH
                    ya = yaspool.tile([128, YB, 258], BF16, name="ya_s",
                                      tag="yas")
                    src = (yap_d if pos else yan_d)[
                        :, cl0 * 258: (cl0 + YB) * 258
                    ].rearrange("p (c f) -> p c f", c=YB)
                    nc.sync.dma_start(ya[:], src)
                    for j in range(YB):
                        if t == 0:
                            ict = icts[0] if pos else icts[1]
                            isl = ict[:, cl0 + j: cl0 + j + 1]
                        else:
                            isl = icts[t + 1][:, c0 + j: c0 + j + 1]
                        nc.vector.tensor_scalar_mul(
                            ya[:, j, 0:256], ya[:, j, 0:256], isl
                        )
                        nc.vector.tensor_copy(ya[:, j, 256:257], isl)
                    return ya

                def drain_one(t, i):
                    pp, pn = psums[0][i], psums[1][i]
                    # stat columns to SBUF first (only one PSUM read per op)
                    sn = drain.tile([128, 2], F32, name="sn", tag="sn")
                    nc.vector.tensor_copy(sn[:], pn[:, 256:258])
                    sp = drain.tile([128, 2], F32, name="sp", tag="sp")
                    nc.vector.tensor_copy(sp[:], pp[:, 256:258])
                    st = drain.tile([128, 1], F32, name="st", tag="st")
                    nc.vector.tensor_tensor(
                        st[:], sn[:, 1:2], sp[:, 1:2], ALU.add
                    )
                    rinv = drain.tile([128, 1], F32, name="rinv", tag="rinv")
                    nc.vector.reciprocal(rinv[:], st[:])
                    af = drain.tile([128, 1], F32, name="af", tag="af")
                    nc.vector.tensor_tensor(
                        af[:], sn[:, 0:1], rinv[:], ALU.mult
                    )
                    bfn = drain.tile([128, 1], F32, name="bfn", tag="bfn")
                    nc.vector.scalar_tensor_tensor(
                        bfn[:], sp[:, 0:1], -1.0, rinv[:],
                        ALU.mult, ALU.mult,
                    )
                    nc.vector.scalar_tensor_tensor(
                        V_sb[:, i, :], pp[:, 0:D], af[:], V_sb[:, i, :],
                        ALU.mult, ALU.add,
                    )
                    nc.vector.scalar_tensor_tensor(
                        V_sb[:, i, :], pn[:, 0:D], bfn[:], V_sb[:, i, :],
                        ALU.mult, ALU.add,
                    )

                def mm_temp(t, kp_sb):
                    for c0 in range(0, NCH, YB):
                        pos = c0 < HCH
                        ya = ya_batch(t, c0)
                        for j in range(YB):
                            c = c0 + j
                            for i in range(ISUB):
                                nc.tensor.matmul(
                                    psums[0 if pos else 1][i][:],
                                    kp_sb[:, c, i * 128: (i + 1) * 128],
                                    ya[:, j, :],
                                    start=(c == 0 or c == HCH),
                                    stop=(c == HCH - 1 or c == NCH - 1),
                                )
                    for i in range(ISUB):
                        drain_one(t, i)

                mm_temp(0, e1_sb)
                mm_temp(1, e2_sb)

                # e4 materialized into e1's slot (free after mm_temp(0))
                EG = 8
                e4_sb = pbig.tile([128, NCH, M], BF16, name="e4_sb",
                                  tag="slotB")
                if e4mat_engine == "dve":
                    for g0 in range(0, NCH, EG):
                        nc.vector.tensor_tensor(
                            e4_sb[:, g0: g0 + EG, :],
                            e2_sb[:, g0: g0 + EG, :],
                            e2_sb[:, g0: g0 + EG, :],
                            ALU.mult,
                        )
                else:
                    for g0 in range(0, NCH, EG):
                        nc.scalar.activation(
                            e4_sb[:, g0: g0 + EG, :],
                            e2_sb[:, g0: g0 + EG, :],
                            AF.Square,
                        )
                mm_temp(2, e4_sb)

            # ---- loss ----
            vsq = stats.tile([128, ISUB, D], F32, name="vsq")
            nc.scalar.activation(
                vsq[:], V_sb[:], AF.Square, accum_out=lout[:]
            )
            nc.sync.dma_start(loss_d[:], lout[:])

    nc.compile()
    return nc


def prepare_inputs(x, y_pos, y_neg, cores=CORES):
    """Host-side input prep: shard, transpose, cast, norms, masks."""
    x = np.asarray(x, dtype=np.float32)
    y_pos = np.asarray(y_pos, dtype=np.float32)
    y_neg = np.asarray(y_neg, dtype=np.float32)
    N, D = x.shape
    M = N // cores
    KCH = D // 128
    HCH = N // 128
    WIN = 128 // cores
    bf = ml_dtypes.bfloat16

    def yT_mat(y):
        # [128, KCH*N]: [p, k*N + n] = y[n, k*128 + p]
        yt = np.ascontiguousarray(y.T).astype(bf)      # [D, N]
        return yt.reshape(KCH, 128, N).transpose(1, 0, 2).reshape(128, KCH * N)

    def ya_mat(y):
        # [128, HCH*258]: [p, c*258 + j] = aug[c*128 + p, j]
        a = np.zeros((N, 258), dtype=bf)
        a[:, :D] = y.astype(bf)
        a[:, 256] = bf(1.0)   # -> rn/rp (gets the 1/sqrt(c) scaling)
        a[:, 257] = bf(1.0)   # -> r_i (stays raw)
        return np.ascontiguousarray(
            a.reshape(HCH, 128, 258).transpose(1, 0, 2).reshape(128, HCH * 258)
        )

    def yx_mat(y):
        s = (y * y).sum(axis=1).astype(np.float32)
        hi = s.astype(bf)
        lo = (s - hi.astype(np.float32)).astype(bf)
        m = np.zeros((128, N), dtype=bf)
        m[0] = bf(1.0)
        m[1] = bf(1.0)
        m[2] = hi
        m[3] = lo
        return m

    shared = {
        "yTp": yT_mat(y_pos),
        "yTn": yT_mat(y_neg),
        "yap": ya_mat(y_pos),
        "yan": ya_mat(y_neg),
        "yxp": yx_mat(y_pos),
        "yxn": yx_mat(y_neg),
        "ones128": np.ones((128, 128), dtype=np.float32),
    }
    in_maps = []
    for c in range(cores):
        xs = x[c::cores]                               # [M, D]
        xt2 = np.ascontiguousarray((-2.0 * xs).T).astype(bf)   # [D, M]
        xt2 = xt2.reshape(KCH, 128, M).transpose(1, 0, 2).reshape(128, KCH * M)
        sqx = (xs * xs).sum(axis=1).astype(np.float32)
        hi = sqx.astype(bf)
        lo = (sqx - hi.astype(np.float32)).astype(bf)
        xse = np.zeros((128, M), dtype=bf)
        xse[0] = hi
        xse[1] = lo
        xse[2] = bf(1.0)
        xse[3] = bf(1.0)
        poison = np.zeros((128, WIN), dtype=np.float32)
        for q in range(WIN):
            poison[c + cores * q, q] = POISON
        m = dict(shared)
        m["xT2"] = xt2
        m["xse"] = xse
        m["poison"] = poison
        in_maps.append(m)
    return in_maps


_CACHED = {}


def _get_nc(cores, N, D):
    key = (cores, N, D)
    if key not in _CACHED:
        _CACHED[key] = build(cores, N, D)
    return _CACHED[key]


def kernel(x, y_pos, y_neg, _trace=False, _tracekw=None):
    x = np.asarray(x)
    N, D = x.shape
    nc = _get_nc(CORES, N, D)
    in_maps = prepare_inputs(x, y_pos, y_neg, CORES)
    kw = dict(_tracekw or {})
    res = run_bass_kernel_spmd(
        nc, in_maps, core_ids=list(range(CORES)), trace=_trace, **kw
    )
    total = sum(float(res.results[c]["losspart"].sum()) for c in range(CORES))
    loss = np.float32(total / (N * D))
    out = np.array(loss, dtype=np.float32)
    if _trace:
        return out, res
    return out


if __name__ == "__main__":
    rng = np.random.default_rng(0)
    N, D = N_FULL, D_FULL
    x = rng.standard_normal((N, D)).astype(np.float32)
    yp = rng.standard_normal((N, D)).astype(np.float32)
    yn = rng.standard_normal((N, D)).astype(np.float32)
    print("loss:", kernel(x, yp, yn))


# revision 7
# speedup vs baseline: 1.1660x; 1.0456x over previous
"""Trainium2 Bass kernel for the DriftingPolicy loss (8-core SPMD), v2.

Math (identical to the baseline / reference):
  For T in {0.2, 0.1, 0.05} = 0.2 / t_hat, t_hat in {1, 2, 4}:
    K_t[n, i] = exp(-t_hat * d[n, i] * N^2 / (0.2 * S)),  S = global sum(d_pos)
    c_n^t  = global column sums of K_t (over all rows i)   -> all-reduce
    K'_t   = K_t / sqrt(c^t);  rn_i = sum_neg K'_t, rp_i = sum_pos K'_t,
    r_i    = sum_all K_t (raw)
    V_i   += (rn_i/r_i) * (K'_pos @ y_pos)_i - (rp_i/r_i) * (K'_neg @ y_neg)_i
  loss = mean(V^2)

v2 restructure vs baseline (~375us -> ~335us measured; most of the
remaining span is cross-core launch skew absorbed at the first collective):
  * Chunk-granular pipeline; the t_hat=1 column-sum AllReduce is split
    pos/neg so the second-phase matmuls start on the pos half early.
  * sqrt writes f16 into e1_sb and exp runs IN PLACE per chunk: no d
    buffer, and crucially no d-slot WAR chain forcing sqrt/exp interleave
    on ACT (which cost ~40 ACT table reloads at 1.3us each in v1 drafts).
  * Column sums fused into the producers via accum_out: colsum1 in the
    per-chunk ACT exp, colsum2 in the DVE scalar_tensor_tensor that forms
    e2 = e1*e1, colsum4 split ACT(pos)/DVE(neg) over a scratch.
  * ya (augmented y) streamed from DRAM in 8-chunk batches and scaled by
    1/sqrt(c) in a small rotating pool; 1/sqrt(c) computed on DVE with the
    bit-trick rsqrt + 2 Newton steps (ACT Rsqrt/Ln would thrash tables,
    DVE pow fails the ISA check).
  * Scales matmuls emitted after the neg distance matmuls so the in-order
    PE queue is not head-blocked waiting on the mean AllReduce.
  * All host-side layouts are partition-contiguous (>=516B descriptors).
  * V updates via scalar_tensor_tensor accumulation directly into V_sb.

Column-chunk order is POS first (chunks 0..31), NEG second (32..63), so the
mean(dist_pos) all-reduce and the first column-sum all-reduce both fire as
early as possible.
"""

import sys

if "/opt/trn_rl_repo" not in sys.path:
    sys.path.insert(0, "/opt/trn_rl_repo")

import numpy as np
import ml_dtypes

import concourse.bass as bass
import concourse.mybir as mybir
import concourse.tile as tile
from concourse import bacc
from concourse.bass_utils import run_bass_kernel_spmd
from concourse.tile_rust import add_dep_helper

F32 = mybir.dt.float32
F16 = mybir.dt.float16
BF16 = mybir.dt.bfloat16
AF = mybir.ActivationFunctionType
ALU = mybir.AluOpType

CORES = 8
N_FULL = 4096
D_FULL = 256
T_BASE = 0.2
T_HATS = (1.0, 2.0, 4.0)
POISON = 1.0e6

D_DTYPE = F16


def build(cores=CORES, N=N_FULL, D=D_FULL, local_sim=False,
          colsum_engine="gpsimd", e4mat_engine="dve", yas_bufs=12,
          d_bufs=16):
    """Builds the SPMD Bass kernel. Same NEFF runs on all cores."""
    M = N // cores            # local rows per core (512)
    HCH = N // 128            # chunks per half (32)
    NCH = 2 * HCH             # total column chunks (pos then neg) (64)
    KCH = D // 128            # contraction chunks (2)
    WIN = 128 // cores        # poison window width per neg chunk (16)
    ISUB = M // 128           # 128-row output subchunks (4)
    NT = len(T_HATS)
    GA = 2                    # chunks per sqrt/psum group
    assert M % 128 == 0 and D % 128 == 0 and N % 128 == 0 and M <= 512
    assert WIN * HCH == M

    nc = bacc.Bacc(
        "TRN2",
        target_bir_lowering=False,
        debug=False,
        enable_asserts=True,
        num_devices=cores,
    )

    # ---- kernel I/O (all host-prepped, partition-contiguous) ----
    xT2_d = nc.dram_tensor("xT2", [128, KCH * M], BF16, kind="ExternalInput")
    xse_d = nc.dram_tensor("xse", [128, M], BF16, kind="ExternalInput")
    yTp_d = nc.dram_tensor("yTp", [128, KCH * N], BF16, kind="ExternalInput")
    yTn_d = nc.dram_tensor("yTn", [128, KCH * N], BF16, kind="ExternalInput")
    yap_d = nc.dram_tensor("yap", [128, HCH * 258], BF16, kind="ExternalInput")
    yan_d = nc.dram_tensor("yan", [128, HCH * 258], BF16, kind="ExternalInput")
    yxp_d = nc.dram_tensor("yxp", [128, N], BF16, kind="ExternalInput")
    yxn_d = nc.dram_tensor("yxn", [128, N], BF16, kind="ExternalInput")
    poison_d = nc.dram_tensor("poison", [128, WIN], F32, kind="ExternalInput")
    ones_d = nc.dram_tensor("ones128", [128, 128], F32, kind="ExternalInput")
    loss_d = nc.dram_tensor("losspart", [128, 1], F32, kind="ExternalOutput")

    rg = [list(range(cores))]

    def all_reduce(inb, outb):
        if local_sim:
            nc.sync.dma_start(outb[:], inb[:])
        else:
            nc.gpsimd.collective_compute(
                "AllReduce",
                ALU.add,
                replica_groups=rg,
                ins=[inb[:].opt()],
                outs=[outb[:].opt()],
            )

    with tile.TileContext(nc) as tc:
        with (
            tc.tile_pool(name="consts", bufs=1) as consts,
            tc.tile_pool(name="stats", bufs=1) as stats,
            tc.tile_pool(name="dram", bufs=1, space="DRAM") as dram,
            tc.tile_pool(name="pbig", bufs=1) as pbig,
            tc.tile_pool(name="ytpool", bufs=2) as ytpool,
            tc.tile_pool(name="yaspool", bufs=4) as yaspool,
            tc.tile_pool(name="cspool", bufs=3) as cspool,
            tc.tile_pool(name="drain", bufs=4) as drain,
        ):
            # ---- constants ----
            xT2 = consts.tile([128, KCH, M], BF16, name="xT2_sb")
            nc.sync.dma_start(xT2[:], xT2_d[:].rearrange("p (k f) -> p k f", k=KCH))
            xse = consts.tile([128, M], BF16, name="xse_sb")
            nc.sync.dma_start(xse[:], xse_d[:])
            # yx loaded per half into one rotating slot (pos used first)
            yx = []
            for h, src_ in enumerate((yxp_d, yxn_d)):
                t = consts.tile([128, N], BF16, name=f"yx_sb{h}", tag="yx")
                nc.sync.dma_start(t[:], src_[:])
                yx.append(t)
            poisonT = consts.tile([128, WIN], F32, name="poison_sb")
            nc.sync.dma_start(poisonT[:], poison_d[:])
            ones128 = consts.tile([128, 128], F32, name="ones_sb")
            nc.sync.dma_start(ones128[:], ones_d[:])

            # ---- persistent state ----
            dsum = stats.tile([128, HCH // GA], F32, name="dsum")
            scales = stats.tile([128, NT], F32, name="scales")
            # col sums: t0 split pos/neg, t1/t2 whole
            colp = [
                stats.tile([128, HCH], F32, name="colp0a"),
                stats.tile([128, HCH], F32, name="colp0b"),
                stats.tile([128, NCH], F32, name="colp1"),
                stats.tile([128, NCH], F32, name="colp2"),
            ]
            colg = [
                stats.tile([128, HCH], F32, name="colg0a"),
                stats.tile([128, HCH], F32, name="colg0b"),
                stats.tile([128, NCH], F32, name="colg1"),
                stats.tile([128, NCH], F32, name="colg2"),
            ]
            # 1/sqrt(c) per temp: t0 pos, t0 neg, t1, t2
            icts = [
                stats.tile([128, HCH], F32, name="ict0a"),
                stats.tile([128, HCH], F32, name="ict0b"),
                stats.tile([128, NCH], F32, name="ict1"),
                stats.tile([128, NCH], F32, name="ict2"),
            ]
            V_sb = stats.tile([128, ISUB, D], F32, name="V_sb")
            msum = stats.tile([128, 1], F32, name="msum")
            sc_vec = stats.tile([128, NT], F32, name="sc_vec")
            inv_s = stats.tile([1, 1], F32, name="inv_s")
            s_sc = stats.tile([1, 1], F32, name="s_sc")
            dtot = stats.tile([128, 1], F32, name="dtot")
            lout = stats.tile([128, 1], F32, name="lout")

            nc.vector.memset(V_sb[:], 0.0)

            # DRAM bounce buffers for collectives
            mean_in = dram.tile([128, 1], F32, name="mean_in")
            mean_out = dram.tile([128, 1], F32, name="mean_out",
                                 addr_space="Shared")
            col_in = [
                dram.tile([128, HCH], F32, name="col_in0a"),
                dram.tile([128, HCH], F32, name="col_in0b"),
                dram.tile([128, NCH], F32, name="col_in1"),
                dram.tile([128, NCH], F32, name="col_in2"),
            ]
            col_out = [
                dram.tile([128, HCH], F32, name="col_out0a", addr_space="Shared"),
                dram.tile([128, HCH], F32, name="col_out0b", addr_space="Shared"),
                dram.tile([128, NCH], F32, name="col_out1", addr_space="Shared"),
                dram.tile([128, NCH], F32, name="col_out2", addr_space="Shared"),
            ]

            # ---- big tensors ----
            # e1 holds sqrt(d2) first (f16), then exp overwrites it chunk by
            # chunk IN PLACE -- no separate d buffer, no d-slot WAR chain
            # forcing sqrt/exp interleave on ACT.  e4 reuses e1's slot later;
            # e2 has its own slot.
            e1_sb = pbig.tile([128, NCH, M], F16, name="e1_sb", tag="slotB")
            e2_sb = pbig.tile([128, NCH, M], BF16, name="e2_sb", tag="slotC")

            # ================= phase A: distances =================
            sqrt_insts = []
            with tc.tile_pool(name="pa", bufs=3, space="PSUM") as pa, \
                 tc.tile_pool(name="psmall", bufs=1, space="PSUM") as psmall:

                NPART = 4                 # yT loads per half
                PW = N // NPART           # columns per yT load (1024)

                def load_yT(h, part):
                    t = ytpool.tile([128, KCH, PW], BF16, name="yT_sb",
                                    tag="yT")
                    src = (yTp_d, yTn_d)[h]
                    nc.sync.dma_start(
                        t[:],
                        src[:].rearrange("p (k f) -> p k f", k=KCH)[
                            :, :, part * PW: (part + 1) * PW
                        ],
                    )
                    return t

                def do_group(g, yT, part):
                    # chunks [g*GA, (g+1)*GA) within one half
                    c0 = g * GA
                    pos = c0 < HCH
                    ps = pa.tile([128, GA, M], F32, name="ps_d")
                    for j in range(GA):
                        c = c0 + j
                        cl = c if pos else c - HCH       # chunk within half
                        cc = cl - part * (PW // 128)     # chunk within yT part
                        for k in range(KCH):
                            nc.tensor.matmul(
                                ps[:, j, :],
                                yT[:, k, cc * 128: (cc + 1) * 128],
                                xT2[:, k, :],
                                start=(k == 0),
                                stop=False,
                            )
                        nc.tensor.matmul(
                            ps[:, j, :],
                            yx[0 if pos else 1][:, cl * 128: (cl + 1) * 128],
                            xse[:],
                            start=False,
                            stop=True,
                        )
                        if not pos:
                            nc.vector.tensor_tensor(
                                ps[:, j, cl * WIN: (cl + 1) * WIN],
                                ps[:, j, cl * WIN: (cl + 1) * WIN],
                                poisonT[:],
                                ALU.add,
                            )
                    si = nc.scalar.activation(
                        e1_sb[:, c0: c0 + GA, :],
                        ps[:],
                        AF.Sqrt,
                        accum_out=dsum[:, g: g + 1] if pos else None,
                    )
                    sqrt_insts.append(si)

                # pos half first: feeds the mean all-reduce
                npart = (PW // 128) // GA  # groups per yT part (4)
                for part in range(NPART):
                    yt = load_yT(0, part)
                    for g in range(part * npart, (part + 1) * npart):
                        do_group(g, yt, part)

                # mean all-reduce (overlaps the neg-half work below)
                nc.vector.reduce_sum(dtot[:], dsum[:],
                                     axis=mybir.AxisListType.X)
                nc.sync.dma_start(mean_in[:], dtot[:])
                all_reduce(mean_in, mean_out)
                nc.sync.dma_start(msum[:], mean_out[:])

                for part in range(NPART):
                    yt = load_yT(1, part)
                    for g in range(HCH // GA + part * npart,
                                   HCH // GA + (part + 1) * npart):
                        do_group(g, yt, part)

                # scales from the global mean.  Emitted AFTER the neg
                # distance matmuls: the ps1 matmul waits ~20us on the mean
                # all-reduce, and the in-order PE queue would head-block
                # every later matmul behind it.
                ps1 = psmall.tile([1, 1], F32, name="ps1")
                nc.tensor.matmul(ps1[:], msum[:], ones128[:, 0:1],
                                 start=True, stop=True)
                nc.vector.reciprocal(inv_s[:], ps1[:])
                nc.vector.memset(sc_vec[:], 0.0)
                for t, th in enumerate(T_HATS):
                    # local_sim: collectives are identity, so the "global"
                    # d_pos sum is the 1-core partial; shrink N^2 to match.
                    nn = (N * N) // cores if local_sim else N * N
                    coef = -th * nn / T_BASE
                    nc.vector.tensor_scalar_mul(
                        sc_vec[0:1, t: t + 1], inv_s[0:1, 0:1], coef
                    )
                psb = psmall.tile([128, NT], F32, name="psb")
                nc.tensor.matmul(psb[:], ones128[:], sc_vec[0:128, :],
                                 start=True, stop=True)
                nc.scalar.copy(scales[:], psb[:])

            # ============ exp / squares / column sums ============
            I32 = mybir.dt.int32

            def rsqrt_dve(dst, src, W):
                """dst = src^-0.5 on DVE only: bit-trick seed + 2 Newton
                steps.  Keeps the rsqrt off ACT (table reloads mid-exp) and
                off unsupported ALU ops (pow fails the DVE ISA check)."""
                sh = drain.tile([128, W], I32, name="rs_sh", tag="rs_sh")
                nc.vector.tensor_scalar(
                    sh[:], src.bitcast(I32), 1, None,
                    ALU.logical_shift_right,
                )
                y = drain.tile([128, W], F32, name="rs_y", tag="rs_y")
                nc.vector.tensor_scalar(
                    y[:].bitcast(I32), sh[:], -1, 0x5F3759DF,
                    ALU.mult, ALU.add,
                )
                t1 = drain.tile([128, W], F32, name="rs_t1", tag="rs_t1")
                for _ in range(2):
                    nc.vector.tensor_tensor(t1[:], y[:], y[:], ALU.mult)
                    nc.vector.tensor_tensor(t1[:], t1[:], src, ALU.mult)
                    nc.vector.tensor_scalar(
                        t1[:], t1[:], -0.5, 1.5, ALU.mult, ALU.add
                    )
                    nc.vector.tensor_tensor(y[:], y[:], t1[:], ALU.mult)
                nc.vector.tensor_copy(dst, y[:])

            def launch_ar(idx):
                nc.sync.dma_start(col_in[idx][:], colp[idx][:])
                all_reduce(col_in[idx], col_out[idx])
                nc.sync.dma_start(colg[idx][:], col_out[idx][:])
                W = HCH if idx < 2 else NCH
                rsqrt_dve(icts[idx][:], colg[idx][:], W)

            # ---- streamed, scaled augmented-y batches ----
            # Pre-emitted at specific points so the in-order DVE stream has
            # each temp's 1/sqrt(c) scaling available the moment its
            # AllReduce lands (otherwise ya prep queues behind whatever
            # else DVE is doing and the PE idles 30us+ per temp).
            YB = 8   # ya chunks per DMA batch
            ya_cache = {}

            def ya_batch(t, c0):
                pos = c0 < HCH
                cl0 = c0 if pos else c0 - HCH
                ya = yaspool.tile([128, YB, 258], BF16, name="ya_s",
                                  tag="yas")
                src = (yap_d if pos else yan_d)[
                    :, cl0 * 258: (cl0 + YB) * 258
                ].rearrange("p (c f) -> p c f", c=YB)
                nc.sync.dma_start(ya[:], src)
                for j in range(YB):
                    if t == 0:
                        ict = icts[0] if pos else icts[1]
                        isl = ict[:, cl0 + j: cl0 + j + 1]
                    else:
                        isl = icts[t + 1][:, c0 + j: c0 + j + 1]
                    nc.vector.tensor_scalar_mul(
                        ya[:, j, 0:256], ya[:, j, 0:256], isl
                    )
                    nc.vector.tensor_copy(ya[:, j, 256:257], isl)
                ya_cache[(t, c0)] = ya

            # e1 = exp(scale * d) per chunk on ACT, colsum1 fused via accum;
            # e2 = e1*e1 per chunk on DVE (scalar_tensor_tensor), colsum2
            # fused.  (Pool engine rejects TensorScalarPtr -> no gpsimd.)
            for c in range(NCH):
                cp = colp[0] if c < HCH else colp[1]
                cc = c if c < HCH else c - HCH
                ei = nc.scalar.activation(
                    e1_sb[:, c, :],
                    e1_sb[:, c, :],
                    AF.Exp,
                    bias=0.0,
                    scale=scales[:, 0:1],
                    accum_out=cp[:, cc: cc + 1],
                )
                # NOTE: no ordering edges needed -- with exp running in-place
                # on e1_sb there is no d-slot reuse chain forcing sqrt/exp
                # interleave, and the scheduler orders sqrt* then exp*
                # naturally (verified: 2 table loads total in the BIR).
                nc.vector.scalar_tensor_tensor(
                    e2_sb[:, c, :],
                    e1_sb[:, c, :],
                    1.0,
                    e1_sb[:, c, :],
                    ALU.mult,
                    ALU.mult,
                    accum_out=colp[2][:, c: c + 1],
                )
                if c == HCH - 1:
                    launch_ar(0)
                    for c0 in range(0, HCH, YB):
                        ya_batch(0, c0)
                if c == NCH - 1:
                    launch_ar(1)
                    for c0 in range(HCH, NCH, YB):
                        ya_batch(0, c0)
            launch_ar(2)
            for c0 in range(0, NCH, YB):
                ya_batch(1, c0)

            # e4 column sums into a rotating scratch, all on ACT (idle after
            # the exp stream; Square shares the exp table set so no reloads).
            for c in range(NCH):
                sc4 = cspool.tile([128, M], BF16, name="e4_scr", tag="cs")
                nc.scalar.activation(
                    sc4[:],
                    e2_sb[:, c, :],
                    AF.Square,
                    accum_out=colp[3][:, c: c + 1],
                )
            launch_ar(3)

            # ============ second-phase matmuls ============
            with tc.tile_pool(name="pc", bufs=1, space="PSUM") as pc:
                psums = [
                    [
                        pc.tile([128, 258], F32, name=f"pch{h}_{i}",
                                tag=f"pch{h}_{i}")
                        for i in range(ISUB)
                    ]
                    for h in range(2)
                ]

                def drain_one(t, i):
                    pp, pn = psums[0][i], psums[1][i]
                    # stat columns to SBUF first (only one PSUM read per op)
                    sn = drain.tile([128, 2], F32, name="sn", tag="sn")
                    nc.vector.tensor_copy(sn[:], pn[:, 256:258])
                    sp = drain.tile([128, 2], F32, name="sp", tag="sp")
                    nc.vector.tensor_copy(sp[:], pp[:, 256:258])
                    st = drain.tile([128, 1], F32, name="st", tag="st")
                    nc.vector.tensor_tensor(
                        st[:], sn[:, 1:2], sp[:, 1:2], ALU.add
                    )
                    rinv = drain.tile([128, 1], F32, name="rinv", tag="rinv")
                    nc.vector.reciprocal(rinv[:], st[:])
                    af = drain.tile([128, 1], F32, name="af", tag="af")
                    nc.vector.tensor_tensor(
                        af[:], sn[:, 0:1], rinv[:], ALU.mult
                    )
                    bfn = drain.tile([128, 1], F32, name="bfn", tag="bfn")
                    nc.vector.scalar_tensor_tensor(
                        bfn[:], sp[:, 0:1], -1.0, rinv[:],
                        ALU.mult, ALU.mult,
                    )
                    nc.vector.scalar_tensor_tensor(
                        V_sb[:, i, :], pp[:, 0:D], af[:], V_sb[:, i, :],
                        ALU.mult, ALU.add,
                    )
                    nc.vector.scalar_tensor_tensor(
                        V_sb[:, i, :], pn[:, 0:D], bfn[:], V_sb[:, i, :],
                        ALU.mult, ALU.add,
                    )

                def mm_temp(t, kp_sb):
                    for c0 in range(0, NCH, YB):
                        pos = c0 < HCH
                        ya = ya_cache.pop((t, c0))
                        for j in range(YB):
                            c = c0 + j
                            for i in range(ISUB):
                                nc.tensor.matmul(
                                    psums[0 if pos else 1][i][:],
                                    kp_sb[:, c, i * 128: (i + 1) * 128],
                                    ya[:, j, :],
                                    start=(c == 0 or c == HCH),
                                    stop=(c == HCH - 1 or c == NCH - 1),
                                )
                    for i in range(ISUB):
                        drain_one(t, i)

                # e4 materialized into e1's slot (free after mm_temp(0)) on
                # DVE, emitted FIRST so it sits ahead of the drain chains in
                # the in-order DVE stream; the t2 ya prep follows it so mm2
                # is never gated on DVE queue position.
                EG = 8
                e4_sb = pbig.tile([128, NCH, M], BF16, name="e4_sb",
                                  tag="slotB")
                for g0 in range(0, NCH, EG):
                    nc.vector.tensor_tensor(
                        e4_sb[:, g0: g0 + EG, :],
                        e2_sb[:, g0: g0 + EG, :],
                        e2_sb[:, g0: g0 + EG, :],
                        ALU.mult,
                    )
                for c0 in range(0, NCH, YB):
                    ya_batch(2, c0)

                mm_temp(0, e1_sb)
                mm_temp(1, e2_sb)
                mm_temp(2, e4_sb)

            # ---- loss ----
            vsq = stats.tile([128, ISUB, D], F32, name="vsq")
            nc.scalar.activation(
                vsq[:], V_sb[:], AF.Square, accum_out=lout[:]
            )
            nc.sync.dma_start(loss_d[:], lout[:])

    nc.compile()
    return nc


def prepare_inputs(x, y_pos, y_neg, cores=CORES):
    """Host-side input prep: shard, transpose, cast, norms, masks."""
    x = np.asarray(x, dtype=np.float32)
    y_pos = np.asarray(y_pos, dtype=np.float32)
    y_neg = np.asarray(y_neg, dtype=np.float32)
    N, D = x.shape
    M = N // cores
    KCH = D // 128
    HCH = N // 128
    WIN = 128 // cores
    bf = ml_dtypes.bfloat16

    def yT_mat(y):
        # [128, KCH*N]: [p, k*N + n] = y[n, k*128 + p]
        yt = np.ascontiguousarray(y.T).astype(bf)      # [D, N]
        return yt.reshape(KCH, 128, N).transpose(1, 0, 2).reshape(128, KCH * N)

    def ya_mat(y):
        # [128, HCH*258]: [p, c*258 + j] = aug[c*128 + p, j]
        a = np.zeros((N, 258), dtype=bf)
        a[:, :D] = y.astype(bf)
        a[:, 256] = bf(1.0)   # -> rn/rp (gets the 1/sqrt(c) scaling)
        a[:, 257] = bf(1.0)   # -> r_i (stays raw)
        return np.ascontiguousarray(
            a.reshape(HCH, 128, 258).transpose(1, 0, 2).reshape(128, HCH * 258)
        )

    def yx_mat(y):
        s = (y * y).sum(axis=1).astype(np.float32)
        hi = s.astype(bf)
        lo = (s - hi.astype(np.float32)).astype(bf)
        m = np.zeros((128, N), dtype=bf)
        m[0] = bf(1.0)
        m[1] = bf(1.0)
        m[2] = hi
        m[3] = lo
        return m

    shared = {
        "yTp": yT_mat(y_pos),
        "yTn": yT_mat(y_neg),
        "yap": ya_mat(y_pos),
        "yan": ya_mat(y_neg),
        "yxp": yx_mat(y_pos),
        "yxn": yx_mat(y_neg),
        "ones128": np.ones((128, 128), dtype=np.float32),
    }
    in_maps = []
    for c in range(cores):
        xs = x[c::cores]                               # [M, D]
        xt2 = np.ascontiguousarray((-2.0 * xs).T).astype(bf)   # [D, M]
        xt2 = xt2.reshape(KCH, 128, M).transpose(1, 0, 2).reshape(128, KCH * M)
        sqx = (xs * xs).sum(axis=1).astype(np.float32)
        hi = sqx.astype(bf)
        lo = (sqx - hi.astype(np.float32)).astype(bf)
        xse = np.zeros((128, M), dtype=bf)
        xse[0] = hi
        xse[1] = lo
        xse[2] = bf(1.0)
        xse[3] = bf(1.0)
        poison = np.zeros((128, WIN), dtype=np.float32)
        for q in range(WIN):
            poison[c + cores * q, q] = POISON
        m = dict(shared)
        m["xT2"] = xt2
        m["xse"] = xse
        m["poison"] = poison
        in_maps.append(m)
    return in_maps


_CACHED = {}


def _get_nc(cores, N, D):
    key = (cores, N, D)
    if key not in _CACHED:
        _CACHED[key] = build(cores, N, D)
    return _CACHED[key]


def kernel(x, y_pos, y_neg, _trace=False, _tracekw=None):
    x = np.asarray(x)
    N, D = x.shape
    nc = _get_nc(CORES, N, D)
    in_maps = prepare_inputs(x, y_pos, y_neg, CORES)
    kw = dict(_tracekw or {})
    res = run_bass_kernel_spmd(
        nc, in_maps, core_ids=list(range(CORES)), trace=_trace, **kw
    )
    total = sum(float(res.results[c]["losspart"].sum()) for c in range(CORES))
    loss = np.float32(total / (N * D))
    out = np.array(loss, dtype=np.float32)
    if _trace:
        return out, res
    return out


if __name__ == "__main__":
    rng = np.random.default_rng(0)
    N, D = N_FULL, D_FULL
    x = rng.standard_normal((N, D)).astype(np.float32)
    yp = rng.standard_normal((N, D)).astype(np.float32)
    yn = rng.standard_normal((N, D)).astype(np.float32)
    print("loss:", kernel(x, yp, yn))
